# revision 1
# baseline (speedup 1.0000x reference)
"""CrossAttentionS2T (attn_all_frame=True) as a Bass/Tile kernel on 8 trn2 cores.

Strategy: data-parallel over batch B=8 -> one batch element per NeuronCore.
Per core, all activations live in transposed [feature, token] layout so every
matmul contracts over the partition axis at full 128-wide PE utilization:

  q_in.T [768,1568]   = t_x slice.T + pos (device add)
  s.T    [768, 784]   = s_x slice.T + pos (device add)
  q.T  = (0.125*Wq) @ q_in.T + 0.125*qb      (scale folded into weights: exact)
  k.T  = Wk @ s.T + kb ; v (natural) = s.T.T @ Wv.T + vb
  scores.T[k,q] = k_h.T^T-contraction -> exp (no max-sub; scores are O(1))
  [o_unnorm.T ; denom] = [v_h | 1]^T @ probs.T   (ones column => softmax denom)
  o.T = o_unnorm.T * bcast(1/denom)
  out.T = Wproj @ o.T + pb -> PE-transpose -> natural [1568,768] -> DMA out

Matmul inputs are bitcast to float32r (full fp32 data, 1 cycle/row for moving
free dim >= 256 on trn2 vs 4 cycles/row for plain fp32).
"""

import math
import os
from contextlib import ExitStack

import numpy as np

import concourse.bass as bass
import concourse.mybir as mybir
import concourse.tile as tile
from concourse.bass import ds, ts
from concourse.masks import make_identity

F32 = mybir.dt.float32
F32R = mybir.dt.float32r
AF = mybir.ActivationFunctionType

# problem dims (hardcoded per contract)
B, SPEC, T = 8, 4, 8
AP_, VP, DIM = 196, 196, 768
NH, HD = 12, 64
SCALE = HD ** -0.5
NQ = VP * T          # 1568 q tokens per batch
NK = AP_ * SPEC      # 784 kv tokens per batch
DC = DIM // 128      # 6 contraction chunks
QT, NQT = 392, 4     # q-token tile (moving free dim)
KB, NKB = 112, 7     # k-token block (scores.T partition dim)
VW, NVW = 384, 2     # v feature tile for natural-layout V projection
NCORES = 8


def _r(ap):
    return ap.bitcast(F32R)


def _emit(ctx, tc, outs, ins):
    nc = tc.nc
    (txT, sxT, posqT, possT, wqT, wkT, wvT, wpT, qb2, kb2, pb2, vbb, ones1) = ins
    (out_d,) = outs

    const = ctx.enter_context(tc.tile_pool(name="const", bufs=1))
    ident = const.tile([128, 128], F32)
    make_identity(nc, ident)
    qb_t = const.tile([128, DC], F32)
    kb_t = const.tile([128, DC], F32)
    pb_t = const.tile([128, DC], F32)
    vbb_t = const.tile([128, DIM], F32)
    nc.sync.dma_start(qb_t[:], qb2[:])
    nc.sync.dma_start(kb_t[:], kb2[:])
    nc.sync.dma_start(pb_t[:], pb2[:])
    nc.sync.dma_start(vbb_t[:], vbb[:])
    ones_t = const.tile([1, HD], F32)
    nc.sync.dma_start(_r(ones_t[:]), _r(ones1[:]))

    # persistent activations
    pers = ctx.enter_context(tc.tile_pool(name="pers", bufs=1))
    oT = [pers.tile([128, NQ], F32, name=f"oT{c}", tag=f"oT{c}") for c in range(DC)]
    kTt = [pers.tile([128, NK], F32, name=f"kT{c}", tag=f"kT{c}") for c in range(DC)]
    v_t = [pers.tile([KB, NH * (HD + 1)], F32, name=f"v{j}", tag=f"v{j}")
           for j in range(NKB)]
    qTt = [pers.tile([128, NQ], F32, name=f"qT{c}", tag=f"qT{c}") for c in range(DC)]
    wp_t = [pers.tile([128, DIM], F32, name=f"wp{c}", tag=f"wp{c}") for c in range(DC)]
    for c in range(DC):
        nc.sync.dma_start(_r(wp_t[c][:]), _r(wpT[ts(c, 128), :]))

    # PSUM pools: proj (3 banks) + attention qk (3) + o (2) = 8 banks total
    psA = ctx.enter_context(tc.tile_pool(name="psA", bufs=2, space="PSUM"))
    psB = ctx.enter_context(tc.tile_pool(name="psB", bufs=1, space="PSUM"))

    # ---- phase A: s.T build + KV projections ----
    with tc.tile_pool(name="phA", bufs=1) as phA:
        wk_t = [phA.tile([128, DIM], F32, name=f"wk{c}", tag=f"wk{c}")
                for c in range(DC)]
        wv_t = [phA.tile([128, DIM], F32, name=f"wv{c}", tag=f"wv{c}")
                for c in range(DC)]
        sT = [phA.tile([128, NK], F32, name=f"sT{c}", tag=f"sT{c}")
              for c in range(DC)]
        for c in range(DC):
            nc.sync.dma_start(_r(wk_t[c][:]), _r(wkT[ts(c, 128), :]))
            nc.sync.dma_start(_r(wv_t[c][:]), _r(wvT[ts(c, 128), :]))
        for c in range(DC):
            sx_t = phA.tile([128, NK], F32, name="sx_t", tag="ldA", bufs=2)
            nc.sync.dma_start(sx_t[:], sxT[ts(c, 128), :])
            ps_t = phA.tile([128, NK], F32, name="ps_t", tag="ldB", bufs=2)
            nc.sync.dma_start(ps_t[:], possT[ts(c, 128), :])
            nc.vector.tensor_add(_r(sT[c][:]), sx_t[:], ps_t[:])

        # K projection, transposed output layout [kfeat, ktok]
        for f in range(DC):
            for kt in range(2):
                ps = psA.tile([128, QT], F32, name="ps_k", tag="proj")
                for c in range(DC):
                    nc.tensor.matmul(
                        ps[:], _r(wk_t[c][:, ts(f, 128)]),
                        _r(sT[c][:, ts(kt, QT)]),
                        start=(c == 0), stop=(c == DC - 1))
                nc.scalar.activation(_r(kTt[f][:, ts(kt, QT)]), ps[:], AF.Identity,
                                     bias=kb_t[:, ds(f, 1)])

        # V projection, natural layout [ktok, vfeat], +1s column per head
        # (memset is not encodable with an f32r output; round via tensor_copy)
        vinit = phA.tile([KB, NH * (HD + 1)], F32, name="vinit", tag="vinit")
        nc.vector.memset(vinit[:], 1.0)
        for j in range(NKB):
            nc.vector.tensor_copy(_r(v_t[j][:]), vinit[:])
            for w in range(NVW):
                ps = psA.tile([KB, VW], F32, name="ps_v", tag="proj")
                for c in range(DC):
                    nc.tensor.matmul(
                        ps[:], _r(sT[c][:, ts(j, KB)]),
                        _r(wv_t[c][:, ts(w, VW)]),
                        start=(c == 0), stop=(c == DC - 1))
                for hh in range(6):
                    h = w * 6 + hh
                    nc.vector.tensor_add(
                        _r(v_t[j][:, ds(h * (HD + 1), HD)]),
                        ps[:, ts(hh, HD)],
                        vbb_t[0:KB, ds(w * VW + hh * HD, HD)])

    # ---- phase B: q_in.T build + Q projection (streamed per q-tile) ----
    with tc.tile_pool(name="phB", bufs=1) as phB:
        wq_t = [phB.tile([128, DIM], F32, name=f"wq{c}", tag=f"wq{c}")
                for c in range(DC)]
        for c in range(DC):
            nc.sync.dma_start(_r(wq_t[c][:]), _r(wqT[ts(c, 128), :]))
        for qt in range(NQT):
            qins = []
            for c in range(DC):
                tx_t = phB.tile([128, QT], F32, name="tx_t", tag="ldq", bufs=3)
                nc.gpsimd.dma_start(tx_t[:], txT[ts(c, 128), ts(qt, QT)])
                pq_t = phB.tile([128, QT], F32, name="pq_t", tag="ldp", bufs=3)
                nc.gpsimd.dma_start(pq_t[:], posqT[ts(c, 128), ts(qt, QT)])
                qin_c = phB.tile([128, QT], F32, name="qin", tag="qin", bufs=2 * DC)
                nc.vector.tensor_add(_r(qin_c[:]), tx_t[:], pq_t[:])
                qins.append(qin_c)
            for f in range(DC):
                ps = psA.tile([128, QT], F32, name="ps_q", tag="proj")
                for c in range(DC):
                    nc.tensor.matmul(
                        ps[:], _r(wq_t[c][:, ts(f, 128)]), _r(qins[c][:]),
                        start=(c == 0), stop=(c == DC - 1))
                nc.scalar.activation(_r(qTt[f][:, ts(qt, QT)]), ps[:], AF.Identity,
                                     bias=qb_t[:, ds(f, 1)])

    # ---- phase C: attention ----
    with tc.tile_pool(name="phC", bufs=1) as phC:
        for h in range(NH):
            ch, off = h // 2, (h % 2) * HD
            for qt in range(NQT):
                q_ap = qTt[ch][ds(off, HD), ts(qt, QT)]
                o_ps = psB.tile([HD + 1, QT], F32, name="o_ps", tag="o", bufs=3)
                probs = []
                for j in range(NKB):
                    s_ps = psB.tile([KB, QT], F32, name="s_ps", tag="qk", bufs=3)
                    nc.tensor.matmul(s_ps[:],
                                     _r(kTt[ch][ds(off, HD), ts(j, KB)]),
                                     _r(q_ap), start=True, stop=True)
                    p_t = phC.tile([KB, QT], F32, name="p_t", tag="probs", bufs=9)
                    nc.scalar.activation(_r(p_t[:]), s_ps[:], AF.Exp)
                    probs.append(p_t)
                for j in range(NKB):
                    nc.tensor.matmul(o_ps[:],
                                     _r(v_t[j][:, ds(h * (HD + 1), HD + 1)]),
                                     _r(probs[j][:]),
                                     start=(j == 0), stop=(j == NKB - 1))
                r1 = phC.tile([1, QT], F32R, name="r1", tag="r1", bufs=2)
                with nc.allow_low_precision(reason="f32r recip for bcast"):
                    nc.vector.reciprocal(r1[:], o_ps[ds(HD, 1), :])
                rb_ps = psB.tile([HD, QT], F32, name="rb_ps", tag="qk", bufs=3)
                nc.tensor.matmul(rb_ps[:], _r(ones_t[:]), r1[:],
                                 start=True, stop=True)
                rb = phC.tile([HD, QT], F32, name="rb", tag="rb", bufs=2)
                nc.vector.tensor_copy(rb[:], rb_ps[:])
                nc.vector.tensor_mul(_r(oT[ch][ds(off, HD), ts(qt, QT)]),
                                     o_ps[0:HD, :], rb[:])

    # ---- phase D: output projection + transpose to natural + DMA out ----
    with tc.tile_pool(name="phD", bufs=1) as phD:
        outT = [phD.tile([128, NQ], F32, name=f"outT{c}", tag=f"outT{c}")
                for c in range(DC)]
        for f in range(DC):
            for qt in range(NQT):
                ps = psA.tile([128, QT], F32, name="ps_o", tag="proj")
                for c in range(DC):
                    nc.tensor.matmul(
                        ps[:], _r(wp_t[c][:, ts(f, 128)]),
                        _r(oT[c][:, ts(qt, QT)]),
                        start=(c == 0), stop=(c == DC - 1))
                nc.scalar.activation(outT[f][:, ts(qt, QT)], ps[:], AF.Identity,
                                     bias=pb_t[:, ds(f, 1)])
        nblk = math.ceil(NQ / 128)  # 13 blocks: 12x128 + 32
        for qb in range(nblk):
            qw = min(128, NQ - qb * 128)
            o_nat = phD.tile([128, DIM], F32, name="o_nat", tag="onat", bufs=2)
            for f in range(DC):
                tp = psA.tile([128, 128], F32, name="tp", tag="proj")
                nc.tensor.transpose(tp[0:qw, :], outT[f][:, ds(qb * 128, qw)],
                                    ident[:])
                nc.vector.tensor_copy(o_nat[0:qw, ts(f, 128)], tp[0:qw, :])
            nc.sync.dma_start(out_d[ds(qb * 128, qw), :], o_nat[0:qw, :])


def build_program():
    from concourse import bacc
    from concourse.compiler_utils import get_compiler_flags, set_compiler_flags
    flags = [f.replace("--enable-ldw-opt=false", "--enable-ldw-opt=true")
             for f in get_compiler_flags()]
    set_compiler_flags(flags)
    nc = bacc.Bacc("TRN2", target_bir_lowering=False, debug=False,
                   num_devices=NCORES)
    mk = lambda name, shape, out=False: nc.dram_tensor(
        name, shape, F32, kind="ExternalOutput" if out else "ExternalInput").ap()
    ins = [
        mk("txT", [DIM, NQ]), mk("sxT", [DIM, NK]),
        mk("posqT", [DIM, NQ]), mk("possT", [DIM, NK]),
        mk("wqT", [DIM, DIM]), mk("wkT", [DIM, DIM]),
        mk("wvT", [DIM, DIM]), mk("wpT", [DIM, DIM]),
        mk("qb2", [128, DC]), mk("kb2", [128, DC]), mk("pb2", [128, DC]),
        mk("vbb", [128, DIM]), mk("ones1", [1, HD]),
    ]
    outs = [mk("out", [NQ, DIM], out=True)]
    with tile.TileContext(nc) as tc:
        with ExitStack() as ctx:
            _emit(ctx, tc, outs, ins)
    nc.compile()
    return nc


def host_prep(inputs):
    """Host-side layout marshalling: slice per core, transpose to
    [feature, token], fold the attention scale into Wq (exact: 0.125 = 2^-3),
    pre-broadcast positional sums and biases."""
    f32 = np.float32
    g = {k: np.asarray(v, dtype=f32) for k, v in inputs.items()}
    t_pat = g["t_x"][1:]                      # (VP, B*T, D)
    s_x = g["s_x"]                            # (AP, B*SPEC, D)

    posq = (g["vmae_space_pos"][:, None, :] + g["vmae_temporal_pos"][None, :, :])
    posq = np.ascontiguousarray(posq.reshape(NQ, DIM).T)          # (D, NQ)
    poss = (g["clip_space_pos"][:, None, :] + g["clip_temporal_pos"][None, :, :])
    poss = np.ascontiguousarray(poss.reshape(NK, DIM).T)          # (D, NK)

    wqT = np.ascontiguousarray((SCALE * g["Wq"]).T)
    wkT = np.ascontiguousarray(g["Wkv"][:DIM].T)
    wvT = np.ascontiguousarray(g["Wkv"][DIM:].T)
    wpT = np.ascontiguousarray(g["Wproj"].T)
    qb2 = np.ascontiguousarray((SCALE * g["q_bias"]).reshape(DC, 128).T)
    kb2 = np.ascontiguousarray(g["kv_bias"][:DIM].reshape(DC, 128).T)
    pb2 = np.ascontiguousarray(g["proj_bias"].reshape(DC, 128).T)
    vbb = np.ascontiguousarray(np.tile(g["kv_bias"][DIM:], (128, 1)))

    shared = dict(posqT=posq, possT=poss, wqT=wqT, wkT=wkT, wvT=wvT, wpT=wpT,
                  qb2=qb2, kb2=kb2, pb2=pb2, vbb=vbb,
                  ones1=np.ones((1, HD), dtype=f32))
    in_maps = []
    for b in range(B):
        txT = np.ascontiguousarray(
            t_pat[:, b * T:(b + 1) * T, :].reshape(NQ, DIM).T)
        sxT = np.ascontiguousarray(
            s_x[:, b * SPEC:(b + 1) * SPEC, :].reshape(NK, DIM).T)
        in_maps.append(dict(txT=txT, sxT=sxT, **shared))
    return in_maps


def host_finish(results, t_x):
    o = np.stack([results[b]["out"] for b in range(B)])   # (B, NQ, D)
    o = o.reshape(B, VP, T, DIM).transpose(1, 0, 2, 3).reshape(VP, B * T, DIM)
    return np.concatenate([np.asarray(t_x, dtype=np.float32)[0:1], o], axis=0)


_NC = None


def kernel(**inputs):
    global _NC
    from concourse.bass_utils import run_bass_kernel_spmd
    if _NC is None:
        _NC = build_program()
    in_maps = host_prep(inputs)
    res = run_bass_kernel_spmd(_NC, in_maps, list(range(NCORES)))
    return host_finish(res.results, inputs["t_x"])



# revision 13
# speedup vs baseline: 2.4052x; 2.4052x over previous
"""CrossAttentionS2T (attn_all_frame=True) as a Bass/Tile kernel on 8 trn2 cores.

Strategy: data-parallel over batch B=8 -> one batch element per NeuronCore.
All matmul operands are bf16 (fp32 PSUM accumulation). K<128-contraction
matmuls run at the full 1 cycle/row rate in bf16 (fp32r needs K=128), which
is what makes the attention phase fast.

Per core:
  qinT [768,1568] = (t_x slice + pos).T   (pos folded on host, bf16)
  sinT [768, 784] = (s_x slice + pos).T
  qT = (scale*Wq) @ qinT   kT = Wk @ sinT          (transposed layouts)
  v   = sinT.T @ Wv.T interleaved [ktok, 12*(64+1)] with a ones column per
        head (ones column => softmax denominator for free)
  scores.T[k,q] per head pair packed in one 2-bank PSUM tile; both heads'
  QK matmuls occupy disjoint PE row strips (rows 0-63 / 64-127) and run
  concurrently; one EXP activation covers both heads.
  o_unnorm.T[65,392] = [v_h | 1]^T @ probs.T ; denominators for all 12 heads
  of a q-tile are gathered by SBUF-SBUF DMA and hit with one batched
  approx-reciprocal; PE broadcasts recip rows ([1,64] ones matmul) and DVE
  multiplies.
  out = oT.T @ Wp.T + b computed directly in natural [q,768] layout (the
  normalized oT chunks are the stationary operand), so no output transpose.

The qt-blocks are software-pipelined: Q-projection f-blocks of qt+1 and the
output-projection rows of qt are interleaved between attention head-pair
blocks, keeping PE busy while the Scalar engine (EXP) is the bottleneck.
"""

import math
import os
from contextlib import ExitStack

import numpy as np

import concourse.bass as bass
import concourse.mybir as mybir
import concourse.tile as tile
from concourse.bass import ds, ts

F32 = mybir.dt.float32
BF16 = mybir.dt.bfloat16
AF = mybir.ActivationFunctionType

# problem dims (hardcoded per contract)
B, SPEC, T = 8, 4, 8
AP_, VP, DIM = 196, 196, 768
NH, HD = 12, 64
NP = NH // 2          # 6 head pairs
SCALE = HD ** -0.5
NQ = VP * T           # 1568 q tokens per batch
NK = AP_ * SPEC       # 784 kv tokens per batch
DC = DIM // 128       # 6 contraction chunks
QT, NQT = 392, 4      # q-token tile
KB, NKB = 112, 7      # k-token block (scores.T partition dim)
VW, NVW = 384, 2      # v feature tile for natural-layout V projection
NCORES = 8


def _emit(ctx, tc, outs, ins):
    nc = tc.nc
    (qinT, sinT, wqT, wkT, wvT, wpT, qb2, kb2, vbb, pbb, ones1) = ins
    (out_d,) = outs

    const = ctx.enter_context(tc.tile_pool(name="const", bufs=1))
    qb_t = const.tile([128, DC], F32)
    kb_t = const.tile([128, DC], F32)
    vbb_t = const.tile([128, NH, HD + 1], BF16)
    pbb_t = const.tile([128, DIM], BF16)
    ones_t = const.tile([1, HD], BF16)
    nc.sync.dma_start(qb_t[:], qb2[:])
    nc.sync.dma_start(kb_t[:], kb2[:])
    nc.sync.dma_start(vbb_t[:], vbb[:])
    nc.sync.dma_start(pbb_t[:], pbb[:])
    nc.sync.dma_start(ones_t[:], ones1[:])

    pers = ctx.enter_context(tc.tile_pool(name="pers", bufs=1))
    wk_t = [pers.tile([128, DIM], BF16, name=f"wk{c}", tag=f"wk{c}")
            for c in range(DC)]
    wv_t = [pers.tile([128, DIM], BF16, name=f"wv{c}", tag=f"wv{c}")
            for c in range(DC)]
    wq_t = [pers.tile([128, DIM], BF16, name=f"wq{c}", tag=f"wq{c}")
            for c in range(DC)]
    wp_t = [pers.tile([128, DIM], BF16, name=f"wp{c}", tag=f"wp{c}")
            for c in range(DC)]
    qin_t = [pers.tile([128, NQ], BF16, name=f"qin{c}", tag=f"qin{c}")
             for c in range(DC)]
    kT = [pers.tile([128, NK], BF16, name=f"kT{c}", tag=f"kT{c}")
          for c in range(DC)]
    qT = [pers.tile([128, NQ], BF16, name=f"qT{c}", tag=f"qT{c}")
          for c in range(DC)]
    v_t = [pers.tile([KB, NH, HD + 1], BF16, name=f"v{j}", tag=f"v{j}")
           for j in range(NKB)]
    oT = [pers.tile([128, NQ], BF16, name=f"oT{c}", tag=f"oT{c}")
          for c in range(DC)]
    den_b = pers.tile([NH, QT], BF16, name="den_b", tag="den_b")
    den_f = pers.tile([NH, QT], F32, name="den_f", tag="den_f")
    rcp_f = pers.tile([NH, QT], F32, name="rcp_f", tag="rcp_f")
    rcp_s = pers.tile([NH, QT], F32, name="rcp_s", tag="rcp_s")
    rcp_b = pers.tile([NH, QT], BF16, name="rcp_b", tag="rcp_b")
    # base-partition-0 staging for the broadcast matmul rhs (packed in free)
    rbi_t = pers.tile([1, NH * QT], BF16, name="rbi", tag="rbi")

    # PSUM: sc 2x(2 banks) + o 2x(1) + proj 2x(1) = 8 banks
    ps = ctx.enter_context(tc.tile_pool(name="ps", bufs=1, space="PSUM"))

    def proj_ps():
        return ps.tile([128, 512], F32, name="pp", tag="proj", bufs=2)

    # ---- phase A: K and V projections (s-side tiles in a scoped pool) ----
    with tc.tile_pool(name="phA", bufs=1) as phA:
        sin_t = [phA.tile([128, NK], BF16, name=f"sin{c}", tag=f"sin{c}")
                 for c in range(DC)]
        # input DMAs: K/V weights + s first (phase A), then q-side
        for c in range(DC):
            nc.sync.dma_start(wk_t[c][:], wkT[ts(c, 128), :])
            nc.sync.dma_start(sin_t[c][:], sinT[ts(c, 128), :])
        for c in range(DC):
            nc.sync.dma_start(wv_t[c][:], wvT[ts(c, 128), :])
        for c in range(DC):
            nc.gpsimd.dma_start(wq_t[c][:], wqT[ts(c, 128), :])
            nc.gpsimd.dma_start(qin_t[c][:], qinT[ts(c, 128), :])
        for c in range(DC):
            nc.sync.dma_start(wp_t[c][:], wpT[ts(c, 128), :])

        for f in range(DC):
            for kt in range(2):
                pp = proj_ps()
                for c in range(DC):
                    nc.tensor.matmul(pp[:, 0:QT], wk_t[c][:, ts(f, 128)],
                                     sin_t[c][:, ts(kt, QT)],
                                     start=(c == 0), stop=(c == DC - 1))
                nc.vector.tensor_scalar_add(kT[f][:, ts(kt, QT)], pp[:, 0:QT],
                                            kb_t[:, ds(f, 1)])
        for j in range(NKB):
            nc.vector.memset(v_t[j][:, :, ds(HD, 1)], 1.0)
            for w in range(NVW):
                pv = ps.tile([KB, 6, HD], F32, name="pv", tag="proj", bufs=2)
                for c in range(DC):
                    nc.tensor.matmul(pv[:, :, :], sin_t[c][:, ts(j, KB)],
                                     wv_t[c][:, ts(w, VW)],
                                     start=(c == 0), stop=(c == DC - 1))
                nc.vector.tensor_add(v_t[j][:, ds(w * 6, 6), ds(0, HD)],
                                     pv[:, :, :],
                                     vbb_t[0:KB, ds(w * 6, 6), ds(0, HD)])

    def b_block(qt, f):
        pp = proj_ps()
        for c in range(DC):
            nc.tensor.matmul(pp[:, 0:QT], wq_t[c][:, ts(f, 128)],
                             qin_t[c][:, ts(qt, QT)],
                             start=(c == 0), stop=(c == DC - 1))
        nc.vector.tensor_scalar_add(qT[f][:, ts(qt, QT)], pp[:, 0:QT],
                                    qb_t[:, ds(f, 1)])

    def d_block(qb):
        qw = min(128, NQ - qb * 128)
        osb = pers.tile([128, DIM], BF16, name="osb", tag="osb", bufs=3)
        for f2 in range(2):
            pp = proj_ps()
            for c in range(DC):
                nc.tensor.matmul(pp[0:qw, 0:VW],
                                 oT[c][:, ds(qb * 128, qw)],
                                 wp_t[c][:, ts(f2, VW)],
                                 start=(c == 0), stop=(c == DC - 1))
            nc.vector.tensor_add(osb[0:qw, ts(f2, VW)], pp[0:qw, 0:VW],
                                 pbb_t[0:qw, ts(f2, VW)])
        nc.sync.dma_start(out_d[ds(qb * 128, qw), :], osb[0:qw, :])

    # ---- phase B(0): Q projection for first q-tile ----
    for f in range(DC):
        b_block(0, f)

    # ---- attention, pipelined over q-tiles ----
    with tc.tile_pool(name="phC", bufs=1) as phC:
        d_done = 0
        for qt in range(NQT):
            oU = {}
            for p in range(NP):
                h0 = 2 * p
                # scores for both heads of the pair: disjoint PE row strips,
                # both [112, 392] outputs in one 2-bank psum tile
                probs = []
                for j in range(NKB):
                    sc = ps.tile([KB, 2, 512], F32, name="sc", tag="sc",
                                 bufs=2)
                    for e in range(2):
                        off = e * HD
                        nc.tensor.matmul(
                            sc[:, ds(e, 1), ds(0, QT)],
                            kT[p][ds(off, HD), ts(j, KB)],
                            qT[p][ds(off, HD), ts(qt, QT)],
                            start=True, stop=True)
                    pr = phC.tile([KB, 2, QT], BF16, name="pr", tag="pr",
                                  bufs=8)
                    nc.scalar.activation(pr[:, :, :], sc[:, :, ds(0, QT)],
                                         AF.Exp)
                    probs.append(pr)
                for e in range(2):
                    h = h0 + e
                    po = ps.tile([HD + 1, QT], F32, name="po", tag="o",
                                 bufs=2)
                    for j in range(NKB):
                        nc.tensor.matmul(po[:, :], v_t[j][:, ds(h, 1), :],
                                         probs[j][:, ds(e, 1), :],
                                         start=(j == 0), stop=(j == NKB - 1))
                    oU[h] = phC.tile([HD + 1, QT], BF16, name=f"oU{h}",
                                     tag=f"oU{h}", bufs=2)
                    nc.vector.tensor_copy(oU[h][:], po[:, :])
                if qt < NQT - 1:
                    b_block(qt + 1, p)
            # normalization for this q-tile: gather denominators, one batched
            # reciprocal, then PE broadcast + DVE multiply
            for h in range(NH):
                nc.gpsimd.dma_start(den_b[ds(h, 1), :], oU[h][ds(HD, 1), :])
            nc.vector.tensor_copy(den_f[:], den_b[:])
            nc.vector.reciprocal_approx_accurate(rcp_f[:], den_f[:], rcp_s[:])
            nc.vector.tensor_copy(rcp_b[:], rcp_f[:])
            for h in range(NH):
                nc.gpsimd.dma_start(rbi_t[:, ts(h, QT)], rcp_b[ds(h, 1), :])
            for h in range(NH):
                rb = ps.tile([128, 512], F32, name="rb", tag="proj", bufs=2)
                nc.tensor.matmul(rb[0:HD, 0:QT], ones_t[:],
                                 rbi_t[:, ts(h, QT)],
                                 start=True, stop=True)
                nc.vector.tensor_mul(
                    oT[h // 2][ds((h % 2) * HD, HD), ts(qt, QT)],
                    oU[h][0:HD, :], rb[0:HD, 0:QT])
            # output projection rows fully covered by finished q-tiles
            d_avail = ((qt + 1) * QT) // 128 if qt < NQT - 1 \
                else math.ceil(NQ / 128)
            while d_done < d_avail:
                d_block(d_done)
                d_done += 1


def build_program():
    from concourse import bacc
    from concourse.compiler_utils import get_compiler_flags, set_compiler_flags
    flags = [f.replace("--enable-ldw-opt=false", "--enable-ldw-opt=true")
             for f in get_compiler_flags()]
    set_compiler_flags(flags)
    nc = bacc.Bacc("TRN2", target_bir_lowering=False, debug=False,
                   num_devices=NCORES)

    def mk(name, shape, dtype=BF16, out=False):
        return nc.dram_tensor(
            name, shape, dtype,
            kind="ExternalOutput" if out else "ExternalInput").ap()

    ins = [
        mk("qinT", [DIM, NQ]), mk("sinT", [DIM, NK]),
        mk("wqT", [DIM, DIM]), mk("wkT", [DIM, DIM]),
        mk("wvT", [DIM, DIM]), mk("wpT", [DIM, DIM]),
        mk("qb2", [128, DC], F32), mk("kb2", [128, DC], F32),
        mk("vbb", [128, NH, HD + 1]), mk("pbb", [128, DIM]),
        mk("ones1", [1, HD]),
    ]
    outs = [mk("out", [NQ, DIM], out=True)]
    with tile.TileContext(nc) as tc:
        with ExitStack() as ctx:
            _emit(ctx, tc, outs, ins)
    nc.compile()
    return nc


def host_prep(inputs):
    """Host-side marshalling: per-core slices, transposed bf16 layouts,
    positional sums folded into the activations, attention scale folded into
    Wq (exact: 0.125 = 2^-3)."""
    import ml_dtypes
    bf = ml_dtypes.bfloat16
    f32 = np.float32
    g = {k: np.asarray(v, dtype=f32) for k, v in inputs.items()}
    t_pat = g["t_x"][1:]                      # (VP, B*T, D)
    s_x = g["s_x"]                            # (AP, B*SPEC, D)

    posq = (g["vmae_space_pos"][:, None, :]
            + g["vmae_temporal_pos"][None, :, :]).reshape(NQ, DIM)
    poss = (g["clip_space_pos"][:, None, :]
            + g["clip_temporal_pos"][None, :, :]).reshape(NK, DIM)

    wqT = np.ascontiguousarray((SCALE * g["Wq"]).T.astype(bf))
    wkT = np.ascontiguousarray(g["Wkv"][:DIM].T.astype(bf))
    wvT = np.ascontiguousarray(g["Wkv"][DIM:].T.astype(bf))
    wpT = np.ascontiguousarray(g["Wproj"].T.astype(bf))
    qb2 = np.ascontiguousarray((SCALE * g["q_bias"]).reshape(DC, 128).T)
    kb2 = np.ascontiguousarray(g["kv_bias"][:DIM].reshape(DC, 128).T)
    vbb = np.ones((128, NH, HD + 1), dtype=bf)
    vbb[:, :, :HD] = np.tile(
        g["kv_bias"][DIM:].reshape(NH, HD)[None], (128, 1, 1)).astype(bf)
    pbb = np.ascontiguousarray(
        np.tile(g["proj_bias"], (128, 1)).astype(bf))

    shared = dict(wqT=wqT, wkT=wkT, wvT=wvT, wpT=wpT, qb2=qb2, kb2=kb2,
                  vbb=vbb, pbb=pbb, ones1=np.ones((1, HD), dtype=bf))
    in_maps = []
    for b in range(B):
        qin = t_pat[:, b * T:(b + 1) * T, :].reshape(NQ, DIM) + posq
        sin = s_x[:, b * SPEC:(b + 1) * SPEC, :].reshape(NK, DIM) + poss
        in_maps.append(dict(
            qinT=np.ascontiguousarray(qin.T).astype(bf),
            sinT=np.ascontiguousarray(sin.T).astype(bf),
            **shared))
    return in_maps


def host_finish(results, t_x):
    o = np.stack([np.asarray(results[b]["out"], dtype=np.float32)
                  for b in range(B)])                  # (B, NQ, D)
    o = o.reshape(B, VP, T, DIM).transpose(1, 0, 2, 3).reshape(VP, B * T, DIM)
    return np.concatenate([np.asarray(t_x, dtype=np.float32)[0:1], o], axis=0)


_NC = None


def kernel(**inputs):
    global _NC
    from concourse.bass_utils import run_bass_kernel_spmd
    if _NC is None:
        _NC = build_program()
    in_maps = host_prep(inputs)
    res = run_bass_kernel_spmd(_NC, in_maps, list(range(NCORES)))
    return host_finish(res.results, inputs["t_x"])


# revision 25
# speedup vs baseline: 2.4964x; 1.0379x over previous
"""CrossAttentionS2T (attn_all_frame=True) as a Bass/Tile kernel on 8 trn2 cores.

Strategy: data-parallel over batch B=8 -> one batch element per NeuronCore.
All matmul operands are bf16 (fp32 PSUM accumulation). K<128-contraction
matmuls run at the full 1 cycle/row rate in bf16 (fp32r needs K=128), which
is what makes the attention phase fast.

Per core:
  qinT [768,1568] = (t_x slice + pos).T   (pos folded on host, bf16)
  sinT [768, 784] = (s_x slice + pos).T
  qT = (scale*Wq) @ qinT   kT = Wk @ sinT          (transposed layouts)
  v   = sinT.T @ Wv.T interleaved [ktok, 12*(64+1)] with a ones column per
        head (ones column => softmax denominator for free)
  scores.T[k,q] per head pair packed in one 2-bank PSUM tile; both heads'
  QK matmuls occupy disjoint PE row strips (rows 0-63 / 64-127) and run
  concurrently; one EXP activation covers both heads.
  o_unnorm.T[65,392] = [v_h | 1]^T @ probs.T ; denominators for all 12 heads
  of a q-tile are gathered by SBUF-SBUF DMA and hit with one batched
  approx-reciprocal; PE broadcasts recip rows ([1,64] ones matmul) and DVE
  multiplies.
  out = oT.T @ Wp.T + b computed directly in natural [q,768] layout (the
  normalized oT chunks are the stationary operand), so no output transpose.

The qt-blocks are software-pipelined: Q-projection f-blocks of qt+1 and the
output-projection rows of qt are interleaved between attention head-pair
blocks, keeping PE busy while the Scalar engine (EXP) is the bottleneck.
"""

import math
import os
from contextlib import ExitStack

import numpy as np

import concourse.bass as bass
import concourse.mybir as mybir
import concourse.tile as tile
from concourse.bass import ds, ts

F32 = mybir.dt.float32
BF16 = mybir.dt.bfloat16
AF = mybir.ActivationFunctionType

# problem dims (hardcoded per contract)
B, SPEC, T = 8, 4, 8
AP_, VP, DIM = 196, 196, 768
NH, HD = 12, 64
NP = NH // 2          # 6 head pairs
SCALE = HD ** -0.5
NQ = VP * T           # 1568 q tokens per batch
NK = AP_ * SPEC       # 784 kv tokens per batch
DC = DIM // 128       # 6 contraction chunks
QT, NQT = 392, 4      # q-token tile
KB, NKB = 112, 7      # k-token block (scores.T partition dim)
VW, NVW = 384, 2      # v feature tile for natural-layout V projection
NCORES = 8


def _emit(ctx, tc, outs, ins):
    nc = tc.nc
    (qinT, sinT, wqT, wkT, wvT, wpT, qb2, kb2, vbb, pbb, mask2) = ins
    (out_d,) = outs

    const = ctx.enter_context(tc.tile_pool(name="const", bufs=1))
    qb_t = const.tile([128, DC], F32)
    kb_t = const.tile([128, DC], F32)
    vbb_t = const.tile([128, NH, HD + 1], BF16)
    pbb_t = const.tile([128, DIM], BF16)
    mask2_t = const.tile([2, 128], BF16)
    nc.sync.dma_start(qb_t[:], qb2[:])
    nc.sync.dma_start(kb_t[:], kb2[:])
    nc.sync.dma_start(vbb_t[:], vbb[:])
    nc.sync.dma_start(pbb_t[:], pbb[:])
    nc.sync.dma_start(mask2_t[:], mask2[:])

    pers = ctx.enter_context(tc.tile_pool(name="pers", bufs=1))
    wk_t = [pers.tile([128, DIM], BF16, name=f"wk{c}", tag=f"wk{c}")
            for c in range(DC)]
    wv_t = [pers.tile([128, DIM], BF16, name=f"wv{c}", tag=f"wv{c}")
            for c in range(DC)]
    wq_t = [pers.tile([128, DIM], BF16, name=f"wq{c}", tag=f"wq{c}")
            for c in range(DC)]
    wp_t = [pers.tile([128, DIM], BF16, name=f"wp{c}", tag=f"wp{c}")
            for c in range(DC)]
    qin_t = [pers.tile([128, NQ], BF16, name=f"qin{c}", tag=f"qin{c}")
             for c in range(DC)]
    kT = [pers.tile([128, NK], BF16, name=f"kT{c}", tag=f"kT{c}")
          for c in range(DC)]
    qT = [pers.tile([128, NQ], BF16, name=f"qT{c}", tag=f"qT{c}")
          for c in range(DC)]
    v_t = [pers.tile([KB, NH, HD + 1], BF16, name=f"v{j}", tag=f"v{j}")
           for j in range(NKB)]
    oT = [pers.tile([128, NQ], BF16, name=f"oT{c}", tag=f"oT{c}")
          for c in range(DC)]
    den_b = pers.tile([NH, QT], BF16, name="den_b", tag="den_b")
    den_f = pers.tile([NH, QT], F32, name="den_f", tag="den_f")
    rcp_f = pers.tile([NH, QT], F32, name="rcp_f", tag="rcp_f")
    rcp_s = pers.tile([NH, QT], F32, name="rcp_s", tag="rcp_s")
    rcp_b = pers.tile([NH, QT], BF16, name="rcp_b", tag="rcp_b")
    # base-partition-0 staging for the broadcast matmul rhs (packed in free,
    # one row per head parity so a K=2 matmul broadcasts a whole head pair)
    rbi_t = pers.tile([2, NP * QT], BF16, name="rbi", tag="rbi")

    # PSUM: sc 2x(2 banks) + o 2x(1) + proj 2x(1) = 8 banks
    ps = ctx.enter_context(tc.tile_pool(name="ps", bufs=1, space="PSUM"))

    def proj_ps():
        return ps.tile([128, 512], F32, name="pp", tag="proj", bufs=2)

    # ---- phase A: K and V projections (s-side tiles in a scoped pool) ----
    with tc.tile_pool(name="phA", bufs=1) as phA:
        sin_t = [phA.tile([128, NK], BF16, name=f"sin{c}", tag=f"sin{c}")
                 for c in range(DC)]
        # input DMAs: K/V weights + s first (phase A), then q-side.
        # Spread across engine DMA queues so the loads don't serialize.
        for c in range(DC):
            nc.sync.dma_start(sin_t[c][:], sinT[ts(c, 128), :])
            nc.scalar.dma_start(wk_t[c][:], wkT[ts(c, 128), :])
        for c in range(DC):
            nc.scalar.dma_start(wv_t[c][:], wvT[ts(c, 128), :])
        for c in range(DC):
            nc.gpsimd.dma_start(wq_t[c][:], wqT[ts(c, 128), :])
            nc.gpsimd.dma_start(qin_t[c][:], qinT[ts(c, 128), :])
            nc.sync.dma_start(wp_t[c][:], wpT[ts(c, 128), :])

        for f in range(DC):
            for kt in range(2):
                pp = proj_ps()
                for c in range(DC):
                    nc.tensor.matmul(pp[:, 0:QT], wk_t[c][:, ts(f, 128)],
                                     sin_t[c][:, ts(kt, QT)],
                                     start=(c == 0), stop=(c == DC - 1))
                nc.vector.tensor_scalar_add(kT[f][:, ts(kt, QT)], pp[:, 0:QT],
                                            kb_t[:, ds(f, 1)])
        for j in range(NKB):
            nc.vector.memset(v_t[j][:, :, ds(HD, 1)], 1.0)
            for w in range(NVW):
                pv = ps.tile([KB, 6, HD], F32, name="pv", tag="proj", bufs=2)
                for c in range(DC):
                    nc.tensor.matmul(pv[:, :, :], sin_t[c][:, ts(j, KB)],
                                     wv_t[c][:, ts(w, VW)],
                                     start=(c == 0), stop=(c == DC - 1))
                nc.vector.tensor_add(v_t[j][:, ds(w * 6, 6), ds(0, HD)],
                                     pv[:, :, :],
                                     vbb_t[0:KB, ds(w * 6, 6), ds(0, HD)])

    def b_block(qt, f):
        pp = proj_ps()
        for c in range(DC):
            nc.tensor.matmul(pp[:, 0:QT], wq_t[c][:, ts(f, 128)],
                             qin_t[c][:, ts(qt, QT)],
                             start=(c == 0), stop=(c == DC - 1))
        nc.vector.tensor_scalar_add(qT[f][:, ts(qt, QT)], pp[:, 0:QT],
                                    qb_t[:, ds(f, 1)])

    def d_block(qb):
        qw = min(128, NQ - qb * 128)
        osb = pers.tile([128, DIM], BF16, name="osb", tag="osb", bufs=3)
        for f2 in range(2):
            pp = proj_ps()
            for c in range(DC):
                nc.tensor.matmul(pp[0:qw, 0:VW],
                                 oT[c][:, ds(qb * 128, qw)],
                                 wp_t[c][:, ts(f2, VW)],
                                 start=(c == 0), stop=(c == DC - 1))
            nc.vector.tensor_add(osb[0:qw, ts(f2, VW)], pp[0:qw, 0:VW],
                                 pbb_t[0:qw, ts(f2, VW)])
        nc.sync.dma_start(out_d[ds(qb * 128, qw), :], osb[0:qw, :])

    # ---- phase B(0): Q projection for first q-tile ----
    for f in range(DC):
        b_block(0, f)

    # ---- attention, pipelined over q-tiles ----
    with tc.tile_pool(name="phC", bufs=1) as phC:

        def emit_scores(qt, p):
            """Both heads of the pair in one 2-bank psum tile; the two QK
            matmuls occupy disjoint PE row strips and run concurrently."""
            probs = []
            for j in range(NKB):
                sc = ps.tile([KB, 2, 512], F32, name="sc", tag="sc", bufs=2)
                for e in range(2):
                    off = e * HD
                    nc.tensor.matmul(
                        sc[:, ds(e, 1), ds(0, QT)],
                        kT[p][ds(off, HD), ts(j, KB)],
                        qT[p][ds(off, HD), ts(qt, QT)],
                        start=True, stop=True)
                pr = phC.tile([KB, 2, QT], BF16, name="pr", tag="pr",
                              bufs=14)
                nc.scalar.activation(pr[:, :, :], sc[:, :, ds(0, QT)],
                                     AF.Exp)
                probs.append(pr)
            return probs

        d_done = 0
        blocks = [(qt, p) for qt in range(NQT) for p in range(NP)]
        probs_cur = emit_scores(*blocks[0])
        oU = {}
        for i, (qt, p) in enumerate(blocks):
            # one-block lookahead keeps PE fed while EXP drains this block
            probs_next = emit_scores(*blocks[i + 1]) \
                if i + 1 < len(blocks) else None
            for e in range(2):
                h = 2 * p + e
                po = ps.tile([HD + 1, QT], F32, name="po", tag="o", bufs=2)
                for j in range(NKB):
                    nc.tensor.matmul(po[:, :], v_t[j][:, ds(h, 1), :],
                                     probs_cur[j][:, ds(e, 1), :],
                                     start=(j == 0), stop=(j == NKB - 1))
                oU[h] = phC.tile([HD + 1, QT], BF16, name=f"oU{h}",
                                 tag=f"oU{h}", bufs=2)
                nc.vector.tensor_copy(oU[h][:], po[:, :])
                # start the denominator gather as soon as the row exists
                nc.sync.dma_start(den_b[ds(h, 1), :], oU[h][ds(HD, 1), :])
            probs_cur = probs_next
            if qt < NQT - 1:
                b_block(qt + 1, p)
            if p < NP - 1:
                continue
            # end of q-tile: one batched reciprocal, then a K=2 block-mask
            # matmul broadcasts both heads of a pair at once
            nc.vector.tensor_copy(den_f[:], den_b[:])
            nc.vector.reciprocal_approx_accurate(rcp_f[:], den_f[:], rcp_s[:])
            nc.vector.tensor_copy(rcp_b[:], rcp_f[:])
            for h in range(NH):
                nc.gpsimd.dma_start(rbi_t[ds(h % 2, 1), ts(h // 2, QT)],
                                    rcp_b[ds(h, 1), :])
            for p2 in range(NP):
                rb = ps.tile([128, 512], F32, name="rb", tag="proj", bufs=2)
                nc.tensor.matmul(rb[:, 0:QT], mask2_t[:],
                                 rbi_t[:, ts(p2, QT)],
                                 start=True, stop=True)
                for e in range(2):
                    h = 2 * p2 + e
                    nc.vector.tensor_mul(
                        oT[p2][ds(e * HD, HD), ts(qt, QT)],
                        oU[h][0:HD, :], rb[ds(e * HD, HD), 0:QT])
            # output projection rows fully covered by finished q-tiles
            d_avail = ((qt + 1) * QT) // 128 if qt < NQT - 1 \
                else math.ceil(NQ / 128)
            while d_done < d_avail:
                d_block(d_done)
                d_done += 1


def build_program():
    from concourse import bacc
    from concourse.compiler_utils import get_compiler_flags, set_compiler_flags
    flags = [f.replace("--enable-ldw-opt=false", "--enable-ldw-opt=true")
             for f in get_compiler_flags()]
    set_compiler_flags(flags)
    nc = bacc.Bacc("TRN2", target_bir_lowering=False, debug=False,
                   num_devices=NCORES)

    def mk(name, shape, dtype=BF16, out=False):
        return nc.dram_tensor(
            name, shape, dtype,
            kind="ExternalOutput" if out else "ExternalInput").ap()

    ins = [
        mk("qinT", [DIM, NQ]), mk("sinT", [DIM, NK]),
        mk("wqT", [DIM, DIM]), mk("wkT", [DIM, DIM]),
        mk("wvT", [DIM, DIM]), mk("wpT", [DIM, DIM]),
        mk("qb2", [128, DC], F32), mk("kb2", [128, DC], F32),
        mk("vbb", [128, NH, HD + 1]), mk("pbb", [128, DIM]),
        mk("mask2", [2, 128]),
    ]
    outs = [mk("out", [NQ, DIM], out=True)]
    with tile.TileContext(nc) as tc:
        with ExitStack() as ctx:
            _emit(ctx, tc, outs, ins)
    nc.compile()
    return nc


def host_prep(inputs):
    """Host-side marshalling: per-core slices, transposed bf16 layouts,
    positional sums folded into the activations, attention scale folded into
    Wq (exact: 0.125 = 2^-3)."""
    import ml_dtypes
    bf = ml_dtypes.bfloat16
    f32 = np.float32
    g = {k: np.asarray(v, dtype=f32) for k, v in inputs.items()}
    t_pat = g["t_x"][1:]                      # (VP, B*T, D)
    s_x = g["s_x"]                            # (AP, B*SPEC, D)

    posq = (g["vmae_space_pos"][:, None, :]
            + g["vmae_temporal_pos"][None, :, :]).reshape(NQ, DIM)
    poss = (g["clip_space_pos"][:, None, :]
            + g["clip_temporal_pos"][None, :, :]).reshape(NK, DIM)

    wqT = np.ascontiguousarray((SCALE * g["Wq"]).T.astype(bf))
    wkT = np.ascontiguousarray(g["Wkv"][:DIM].T.astype(bf))
    wvT = np.ascontiguousarray(g["Wkv"][DIM:].T.astype(bf))
    wpT = np.ascontiguousarray(g["Wproj"].T.astype(bf))
    qb2 = np.ascontiguousarray((SCALE * g["q_bias"]).reshape(DC, 128).T)
    kb2 = np.ascontiguousarray(g["kv_bias"][:DIM].reshape(DC, 128).T)
    vbb = np.ones((128, NH, HD + 1), dtype=bf)
    vbb[:, :, :HD] = np.tile(
        g["kv_bias"][DIM:].reshape(NH, HD)[None], (128, 1, 1)).astype(bf)
    pbb = np.ascontiguousarray(
        np.tile(g["proj_bias"], (128, 1)).astype(bf))

    mask2 = np.zeros((2, 128), dtype=bf)
    mask2[0, :HD] = 1
    mask2[1, HD:] = 1
    shared = dict(wqT=wqT, wkT=wkT, wvT=wvT, wpT=wpT, qb2=qb2, kb2=kb2,
                  vbb=vbb, pbb=pbb, mask2=mask2)
    in_maps = []
    for b in range(B):
        qin = t_pat[:, b * T:(b + 1) * T, :].reshape(NQ, DIM) + posq
        sin = s_x[:, b * SPEC:(b + 1) * SPEC, :].reshape(NK, DIM) + poss
        in_maps.append(dict(
            qinT=np.ascontiguousarray(qin.T).astype(bf),
            sinT=np.ascontiguousarray(sin.T).astype(bf),
            **shared))
    return in_maps


def host_finish(results, t_x):
    o = np.stack([np.asarray(results[b]["out"], dtype=np.float32)
                  for b in range(B)])                  # (B, NQ, D)
    o = o.reshape(B, VP, T, DIM).transpose(1, 0, 2, 3).reshape(VP, B * T, DIM)
    return np.concatenate([np.asarray(t_x, dtype=np.float32)[0:1], o], axis=0)


_NC = None


def kernel(**inputs):
    global _NC
    from concourse.bass_utils import run_bass_kernel_spmd
    if _NC is None:
        _NC = build_program()
    in_maps = host_prep(inputs)
    res = run_bass_kernel_spmd(_NC, in_maps, list(range(NCORES)))
    return host_finish(res.results, inputs["t_x"])


# revision 33
# speedup vs baseline: 2.5289x; 1.0130x over previous
"""CrossAttentionS2T (attn_all_frame=True) as a Bass/Tile kernel on 8 trn2 cores.

Strategy: data-parallel over batch B=8 -> one batch element per NeuronCore.
All matmul operands are bf16 (fp32 PSUM accumulation). K<128-contraction
matmuls run at the full 1 cycle/row rate in bf16 (fp32r needs K=128), which
is what makes the attention phase fast.

Per core:
  qinT [768,1568] = (t_x slice + pos).T   (pos folded on host, bf16)
  sinT [768, 784] = (s_x slice + pos).T
  qT = (scale*Wq) @ qinT   kT = Wk @ sinT          (transposed layouts)
  v   = sinT.T @ Wv.T interleaved [ktok, 12*(64+1)] with a ones column per
        head (ones column => softmax denominator for free)
  scores.T[k,q] per head pair packed in one 2-bank PSUM tile; both heads'
  QK matmuls occupy disjoint PE row strips (rows 0-63 / 64-127) and run
  concurrently; one EXP activation covers both heads.
  o_unnorm.T[65,392] = [v_h | 1]^T @ probs.T ; denominators for all 12 heads
  of a q-tile are gathered by SBUF-SBUF DMA and hit with one batched
  approx-reciprocal; PE broadcasts recip rows ([1,64] ones matmul) and DVE
  multiplies.
  out = oT.T @ Wp.T + b computed directly in natural [q,768] layout (the
  normalized oT chunks are the stationary operand), so no output transpose.

The qt-blocks are software-pipelined: Q-projection f-blocks of qt+1 and the
output-projection rows of qt are interleaved between attention head-pair
blocks, keeping PE busy while the Scalar engine (EXP) is the bottleneck.
"""

import math
import os
from contextlib import ExitStack

import numpy as np

import concourse.bass as bass
import concourse.mybir as mybir
import concourse.tile as tile
from concourse.bass import ds, ts

F32 = mybir.dt.float32
BF16 = mybir.dt.bfloat16
AF = mybir.ActivationFunctionType

# problem dims (hardcoded per contract)
B, SPEC, T = 8, 4, 8
AP_, VP, DIM = 196, 196, 768
NH, HD = 12, 64
NP = NH // 2          # 6 head pairs
SCALE = HD ** -0.5
NQ = VP * T           # 1568 q tokens per batch
NK = AP_ * SPEC       # 784 kv tokens per batch
DC = DIM // 128       # 6 contraction chunks
QT, NQT = 392, 4      # q-token tile
KB, NKB = 112, 7      # k-token block (scores.T partition dim)
VW, NVW = 384, 2      # v feature tile for natural-layout V projection
NCORES = 8


def _emit(ctx, tc, outs, ins):
    nc = tc.nc
    (qinT, sinT, wqT, wkT, wvT, wpT, qb2, kb2, vbb, pbb, mask2) = ins
    (out_d,) = outs

    const = ctx.enter_context(tc.tile_pool(name="const", bufs=1))
    qb_t = const.tile([128, DC], F32)
    kb_t = const.tile([128, DC], F32)
    vbb_t = const.tile([128, NH, HD + 1], BF16)
    pbb_t = const.tile([128, DIM], BF16)
    mask2_t = const.tile([2, 128], BF16)

    pers = ctx.enter_context(tc.tile_pool(name="pers", bufs=1))
    # inputs packed [128, DC, X] so each loads in one big-descriptor DMA
    wk_a = pers.tile([128, DC, DIM], BF16, name="wk", tag="wk")
    wv_a = pers.tile([128, DC, DIM], BF16, name="wv", tag="wv")
    wq_a = pers.tile([128, DC, DIM], BF16, name="wq", tag="wq")
    wp_a = pers.tile([128, DC, DIM], BF16, name="wp", tag="wp")
    qin_a = pers.tile([128, DC, NQ], BF16, name="qin", tag="qin")
    kT = [pers.tile([128, NK], BF16, name=f"kT{c}", tag=f"kT{c}")
          for c in range(DC)]
    qT = [pers.tile([128, NQ], BF16, name=f"qT{c}", tag=f"qT{c}")
          for c in range(DC)]
    v_t = [pers.tile([KB, NH, HD + 1], BF16, name=f"v{j}", tag=f"v{j}")
           for j in range(NKB)]
    oT = [pers.tile([128, NQ], BF16, name=f"oT{c}", tag=f"oT{c}")
          for c in range(DC)]
    den_b = pers.tile([NH, QT], BF16, name="den_b", tag="den_b")
    den_f = pers.tile([NH, QT], F32, name="den_f", tag="den_f")
    rcp_f = pers.tile([NH, QT], F32, name="rcp_f", tag="rcp_f")
    rcp_s = pers.tile([NH, QT], F32, name="rcp_s", tag="rcp_s")
    rcp_b = pers.tile([NH, QT], BF16, name="rcp_b", tag="rcp_b")
    # base-partition-0 staging for the broadcast matmul rhs (packed in free,
    # one row per head parity so a K=2 matmul broadcasts a whole head pair)
    rbi_t = pers.tile([2, NP * QT], BF16, name="rbi", tag="rbi")

    # PSUM: sc 2x(2 banks) + o 2x(1) + proj 2x(1) = 8 banks
    ps = ctx.enter_context(tc.tile_pool(name="ps", bufs=1, space="PSUM"))

    def proj_ps():
        return ps.tile([128, 512], F32, name="pp", tag="proj", bufs=2)

    # ---- phase A: K and V projections (s-side tiles in a scoped pool) ----
    with tc.tile_pool(name="phA", bufs=1) as phA:
        sin_a = phA.tile([128, DC, NK], BF16, name="sin", tag="sin")
        # input DMAs, ordered by need time across the three DMA-capable
        # queues (sync/scalar/gpsimd) so phase A is fed first
        nc.sync.dma_start(sin_a[:], sinT[:])
        nc.scalar.dma_start(wk_a[:], wkT[:])
        nc.sync.dma_start(qb_t[:], qb2[:])
        nc.sync.dma_start(kb_t[:], kb2[:])
        nc.sync.dma_start(mask2_t[:], mask2[:])
        nc.scalar.dma_start(wv_a[:], wvT[:])
        nc.scalar.dma_start(vbb_t[:], vbb[:])
        nc.gpsimd.dma_start(wq_a[:], wqT[:])
        nc.gpsimd.dma_start(qin_a[:], qinT[:])
        nc.sync.dma_start(wp_a[:], wpT[:])
        nc.gpsimd.dma_start(pbb_t[:], pbb[:])

        for f in range(DC):
            for kt in range(2):
                pp = proj_ps()
                for c in range(DC):
                    nc.tensor.matmul(pp[:, 0:QT],
                                     wk_a[:, ds(c, 1), ts(f, 128)],
                                     sin_a[:, ds(c, 1), ts(kt, QT)],
                                     start=(c == 0), stop=(c == DC - 1))
                nc.vector.tensor_scalar_add(kT[f][:, ts(kt, QT)], pp[:, 0:QT],
                                            kb_t[:, ds(f, 1)])
        for j in range(NKB):
            nc.vector.memset(v_t[j][:, :, ds(HD, 1)], 1.0)
            for w in range(NVW):
                pv = ps.tile([KB, 6, HD], F32, name="pv", tag="proj", bufs=2)
                for c in range(DC):
                    nc.tensor.matmul(pv[:, :, :],
                                     sin_a[:, ds(c, 1), ts(j, KB)],
                                     wv_a[:, ds(c, 1), ts(w, VW)],
                                     start=(c == 0), stop=(c == DC - 1))
                nc.vector.tensor_add(v_t[j][:, ds(w * 6, 6), ds(0, HD)],
                                     pv[:, :, :],
                                     vbb_t[0:KB, ds(w * 6, 6), ds(0, HD)])

    def b_block(qt, f):
        pp = proj_ps()
        for c in range(DC):
            nc.tensor.matmul(pp[:, 0:QT], wq_a[:, ds(c, 1), ts(f, 128)],
                             qin_a[:, ds(c, 1), ts(qt, QT)],
                             start=(c == 0), stop=(c == DC - 1))
        nc.vector.tensor_scalar_add(qT[f][:, ts(qt, QT)], pp[:, 0:QT],
                                    qb_t[:, ds(f, 1)])

    def d_block(qb):
        qw = min(128, NQ - qb * 128)
        osb = pers.tile([128, DIM], BF16, name="osb", tag="osb", bufs=3)
        for f2 in range(2):
            pp = proj_ps()
            for c in range(DC):
                nc.tensor.matmul(pp[0:qw, 0:VW],
                                 oT[c][:, ds(qb * 128, qw)],
                                 wp_a[:, ds(c, 1), ts(f2, VW)],
                                 start=(c == 0), stop=(c == DC - 1))
            nc.vector.tensor_add(osb[0:qw, ts(f2, VW)], pp[0:qw, 0:VW],
                                 pbb_t[0:qw, ts(f2, VW)])
        nc.sync.dma_start(out_d[ds(qb * 128, qw), :], osb[0:qw, :])

    # ---- phase B(0): Q projection for first q-tile ----
    for f in range(DC):
        b_block(0, f)

    # ---- attention, pipelined over q-tiles ----
    with tc.tile_pool(name="phC", bufs=1) as phC:

        def emit_scores(qt, p):
            """Both heads of the pair in one 2-bank psum tile; the two QK
            matmuls occupy disjoint PE row strips and run concurrently."""
            probs = []
            for j in range(NKB):
                sc = ps.tile([KB, 2, 512], F32, name="sc", tag="sc", bufs=2)
                for e in range(2):
                    off = e * HD
                    nc.tensor.matmul(
                        sc[:, ds(e, 1), ds(0, QT)],
                        kT[p][ds(off, HD), ts(j, KB)],
                        qT[p][ds(off, HD), ts(qt, QT)],
                        start=True, stop=True)
                pr = phC.tile([KB, 2, QT], BF16, name="pr", tag="pr",
                              bufs=14)
                nc.scalar.activation(pr[:, :, :], sc[:, :, ds(0, QT)],
                                     AF.Exp)
                probs.append(pr)
            return probs

        d_done = 0
        blocks = [(qt, p) for qt in range(NQT) for p in range(NP)]
        probs_cur = emit_scores(*blocks[0])
        oU = {}
        for i, (qt, p) in enumerate(blocks):
            # one-block lookahead keeps PE fed while EXP drains this block
            probs_next = emit_scores(*blocks[i + 1]) \
                if i + 1 < len(blocks) else None
            for e in range(2):
                h = 2 * p + e
                po = ps.tile([HD + 1, QT], F32, name="po", tag="o", bufs=2)
                for j in range(NKB):
                    nc.tensor.matmul(po[:, :], v_t[j][:, ds(h, 1), :],
                                     probs_cur[j][:, ds(e, 1), :],
                                     start=(j == 0), stop=(j == NKB - 1))
                oU[h] = phC.tile([HD + 1, QT], BF16, name=f"oU{h}",
                                 tag=f"oU{h}", bufs=2)
                nc.vector.tensor_copy(oU[h][:], po[:, :])
                # start the denominator gather as soon as the row exists
                nc.sync.dma_start(den_b[ds(h, 1), :], oU[h][ds(HD, 1), :])
            probs_cur = probs_next
            if qt < NQT - 1:
                b_block(qt + 1, p)
            if p < NP - 1:
                continue
            # end of q-tile: one batched reciprocal, then a K=2 block-mask
            # matmul broadcasts both heads of a pair at once
            nc.vector.tensor_copy(den_f[:], den_b[:])
            nc.vector.reciprocal_approx_accurate(rcp_f[:], den_f[:], rcp_s[:])
            nc.vector.tensor_copy(rcp_b[:], rcp_f[:])
            for h in range(NH):
                nc.gpsimd.dma_start(rbi_t[ds(h % 2, 1), ts(h // 2, QT)],
                                    rcp_b[ds(h, 1), :])
            for p2 in range(NP):
                rb = ps.tile([128, 512], F32, name="rb", tag="proj", bufs=2)
                nc.tensor.matmul(rb[:, 0:QT], mask2_t[:],
                                 rbi_t[:, ts(p2, QT)],
                                 start=True, stop=True)
                for e in range(2):
                    h = 2 * p2 + e
                    nc.vector.tensor_mul(
                        oT[p2][ds(e * HD, HD), ts(qt, QT)],
                        oU[h][0:HD, :], rb[ds(e * HD, HD), 0:QT])
            # output projection rows fully covered by finished q-tiles
            d_avail = ((qt + 1) * QT) // 128 if qt < NQT - 1 \
                else math.ceil(NQ / 128)
            while d_done < d_avail:
                d_block(d_done)
                d_done += 1


def build_program():
    from concourse import bacc
    from concourse.compiler_utils import get_compiler_flags, set_compiler_flags
    flags = [f.replace("--enable-ldw-opt=false", "--enable-ldw-opt=true")
             for f in get_compiler_flags()]
    set_compiler_flags(flags)
    nc = bacc.Bacc("TRN2", target_bir_lowering=False, debug=False,
                   num_devices=NCORES)

    def mk(name, shape, dtype=BF16, out=False):
        return nc.dram_tensor(
            name, shape, dtype,
            kind="ExternalOutput" if out else "ExternalInput").ap()

    ins = [
        mk("qinT", [128, DC, NQ]), mk("sinT", [128, DC, NK]),
        mk("wqT", [128, DC, DIM]), mk("wkT", [128, DC, DIM]),
        mk("wvT", [128, DC, DIM]), mk("wpT", [128, DC, DIM]),
        mk("qb2", [128, DC], F32), mk("kb2", [128, DC], F32),
        mk("vbb", [128, NH, HD + 1]), mk("pbb", [128, DIM]),
        mk("mask2", [2, 128]),
    ]
    outs = [mk("out", [NQ, DIM], out=True)]
    with tile.TileContext(nc) as tc:
        with ExitStack() as ctx:
            _emit(ctx, tc, outs, ins)
    nc.compile()
    return nc


def host_prep(inputs):
    """Host-side marshalling: per-core slices, transposed bf16 layouts,
    positional sums folded into the activations, attention scale folded into
    Wq (exact: 0.125 = 2^-3)."""
    import ml_dtypes
    bf = ml_dtypes.bfloat16
    f32 = np.float32
    g = {k: np.asarray(v, dtype=f32) for k, v in inputs.items()}
    t_pat = g["t_x"][1:]                      # (VP, B*T, D)
    s_x = g["s_x"]                            # (AP, B*SPEC, D)

    posq = (g["vmae_space_pos"][:, None, :]
            + g["vmae_temporal_pos"][None, :, :]).reshape(NQ, DIM)
    poss = (g["clip_space_pos"][:, None, :]
            + g["clip_temporal_pos"][None, :, :]).reshape(NK, DIM)

    def pack(a_t):
        # [768, X] -> [128, DC, X]: partition-major so one DMA with big
        # per-partition descriptors loads the whole tensor
        x = a_t.shape[1]
        return np.ascontiguousarray(
            a_t.reshape(DC, 128, x).transpose(1, 0, 2)).astype(bf)

    wqT = pack((SCALE * g["Wq"]).T)
    wkT = pack(g["Wkv"][:DIM].T)
    wvT = pack(g["Wkv"][DIM:].T)
    wpT = pack(g["Wproj"].T)
    qb2 = np.ascontiguousarray((SCALE * g["q_bias"]).reshape(DC, 128).T)
    kb2 = np.ascontiguousarray(g["kv_bias"][:DIM].reshape(DC, 128).T)
    vbb = np.ones((128, NH, HD + 1), dtype=bf)
    vbb[:, :, :HD] = np.tile(
        g["kv_bias"][DIM:].reshape(NH, HD)[None], (128, 1, 1)).astype(bf)
    pbb = np.ascontiguousarray(
        np.tile(g["proj_bias"], (128, 1)).astype(bf))

    mask2 = np.zeros((2, 128), dtype=bf)
    mask2[0, :HD] = 1
    mask2[1, HD:] = 1
    shared = dict(wqT=wqT, wkT=wkT, wvT=wvT, wpT=wpT, qb2=qb2, kb2=kb2,
                  vbb=vbb, pbb=pbb, mask2=mask2)
    in_maps = []
    for b in range(B):
        qin = t_pat[:, b * T:(b + 1) * T, :].reshape(NQ, DIM) + posq
        sin = s_x[:, b * SPEC:(b + 1) * SPEC, :].reshape(NK, DIM) + poss
        in_maps.append(dict(
            qinT=pack(qin.T),
            sinT=pack(sin.T),
            **shared))
    return in_maps


def host_finish(results, t_x):
    o = np.stack([np.asarray(results[b]["out"], dtype=np.float32)
                  for b in range(B)])                  # (B, NQ, D)
    o = o.reshape(B, VP, T, DIM).transpose(1, 0, 2, 3).reshape(VP, B * T, DIM)
    return np.concatenate([np.asarray(t_x, dtype=np.float32)[0:1], o], axis=0)


_NC = None


def kernel(**inputs):
    global _NC
    from concourse.bass_utils import run_bass_kernel_spmd
    if _NC is None:
        _NC = build_program()
    in_maps = host_prep(inputs)
    res = run_bass_kernel_spmd(_NC, in_maps, list(range(NCORES)))
    return host_finish(res.results, inputs["t_x"])


# revision 47
# speedup vs baseline: 2.5586x; 1.0117x over previous
"""CrossAttentionS2T (attn_all_frame=True) as a Bass/Tile kernel on 8 trn2 cores.

Strategy: data-parallel over batch B=8 -> one batch element per NeuronCore.
All matmul operands are bf16 (fp32 PSUM accumulation). K<128-contraction
matmuls run at the full 1 cycle/row rate in bf16 (fp32r needs K=128), which
is what makes the attention phase fast.

Per core:
  qinT [768,1568] = (t_x slice + pos).T   (pos folded on host, bf16)
  sinT [768, 784] = (s_x slice + pos).T
  qT = (scale*Wq) @ qinT   kT = Wk @ sinT          (transposed layouts)
  v   = sinT.T @ Wv.T interleaved [ktok, 12*(64+1)] with a ones column per
        head (ones column => softmax denominator for free)
  scores.T[k,q] per head pair packed in one 2-bank PSUM tile; both heads'
  QK matmuls occupy disjoint PE row strips (rows 0-63 / 64-127) and run
  concurrently; one EXP activation covers both heads.
  o_unnorm.T[65,392] = [v_h | 1]^T @ probs.T ; denominators for all 12 heads
  of a q-tile are gathered by SBUF-SBUF DMA and hit with one batched
  approx-reciprocal; PE broadcasts recip rows ([1,64] ones matmul) and DVE
  multiplies.
  out = oT.T @ Wp.T + b computed directly in natural [q,768] layout (the
  normalized oT chunks are the stationary operand), so no output transpose.

The qt-blocks are software-pipelined: Q-projection f-blocks of qt+1 and the
output-projection rows of qt are interleaved between attention head-pair
blocks, keeping PE busy while the Scalar engine (EXP) is the bottleneck.
"""

import math
import os
from contextlib import ExitStack

import numpy as np

import concourse.bass as bass
import concourse.mybir as mybir
import concourse.tile as tile
from concourse.bass import ds, ts

F32 = mybir.dt.float32
BF16 = mybir.dt.bfloat16
AF = mybir.ActivationFunctionType

# problem dims (hardcoded per contract)
B, SPEC, T = 8, 4, 8
AP_, VP, DIM = 196, 196, 768
NH, HD = 12, 64
NP = NH // 2          # 6 head pairs
SCALE = HD ** -0.5
NQ = VP * T           # 1568 q tokens per batch
NK = AP_ * SPEC       # 784 kv tokens per batch
DC = DIM // 128       # 6 contraction chunks
QT, NQT = 392, 4      # q-token tile
KB, NKB = 112, 7      # k-token block (scores.T partition dim)
VW, NVW = 384, 2      # v feature tile for natural-layout V projection
NCORES = 8


def _emit(ctx, tc, outs, ins):
    nc = tc.nc
    (qinT, sinT, wqT, wkT, wvT, wpT, qb2, kb2, vbb, pbb, mask2) = ins
    (out_d,) = outs

    const = ctx.enter_context(tc.tile_pool(name="const", bufs=1))
    qb_t = const.tile([128, DC], F32)
    kb_t = const.tile([128, DC], F32)
    vbb_t = const.tile([128, NH, HD + 1], BF16)
    pbb_t = const.tile([128, DIM], BF16)
    mask2_t = const.tile([2, 128], BF16)

    pers = ctx.enter_context(tc.tile_pool(name="pers", bufs=1))
    # inputs packed [128, DC, X] so each loads in one big-descriptor DMA
    wk_a = pers.tile([128, DC, DIM], BF16, name="wk", tag="wk")
    wv_a = pers.tile([128, DC, DIM], BF16, name="wv", tag="wv")
    wq_a = pers.tile([128, DC, DIM], BF16, name="wq", tag="wq")
    wp_a = pers.tile([128, DC, DIM], BF16, name="wp", tag="wp")
    qin_a = pers.tile([128, NQT, DC, QT], BF16, name="qin", tag="qin")
    kT = [pers.tile([128, NK], BF16, name=f"kT{c}", tag=f"kT{c}")
          for c in range(DC)]
    qT = [pers.tile([128, NQ], BF16, name=f"qT{c}", tag=f"qT{c}")
          for c in range(DC)]
    # per-head stationary padded to 128 cols (64 v + 1 ones + 63 junk) so
    # LDWEIGHTS gets FWL and pipelines; junk cols only write junk PSUM rows
    v_t = [pers.tile([KB, NH, 128], BF16, name=f"v{j}", tag=f"v{j}")
           for j in range(NKB)]
    oT = [pers.tile([128, NQ], BF16, name=f"oT{c}", tag=f"oT{c}")
          for c in range(DC)]
    den_b = pers.tile([NH, QT], BF16, name="den_b", tag="den_b")
    den_f = pers.tile([NH, QT], F32, name="den_f", tag="den_f")
    rcp_f = pers.tile([NH, QT], F32, name="rcp_f", tag="rcp_f")
    rcp_s = pers.tile([NH, QT], F32, name="rcp_s", tag="rcp_s")
    rcp_b = pers.tile([NH, QT], BF16, name="rcp_b", tag="rcp_b")
    # base-partition-0 staging for the broadcast matmul rhs (packed in free,
    # one row per head parity so a K=2 matmul broadcasts a whole head pair)
    rbi_t = pers.tile([2, NP * QT], BF16, name="rbi", tag="rbi")

    # PSUM: sc 2x(2 banks) + o 2x(1) + proj 2x(1) = 8 banks
    ps = ctx.enter_context(tc.tile_pool(name="ps", bufs=1, space="PSUM"))

    def proj_ps():
        return ps.tile([128, 512], F32, name="pp", tag="proj", bufs=2)

    # ---- phase A: K and V projections (s-side tiles in a scoped pool) ----
    with tc.tile_pool(name="phA", bufs=1) as phA:
        sin_a = phA.tile([128, DC, NK], BF16, name="sin", tag="sin")
        # input DMAs: phase-A bytes (sin+wk) split in thirds across all
        # three DMA queues; later-needed tensors queued behind them
        nc.sync.dma_start(qb_t[:], qb2[:])
        nc.sync.dma_start(kb_t[:], kb2[:])
        nc.sync.dma_start(mask2_t[:], mask2[:])
        nc.sync.dma_start(sin_a[:, ds(0, 3), :], sinT[:, ds(0, 3), :])
        nc.scalar.dma_start(wk_a[:, ds(0, 3), :], wkT[:, ds(0, 3), :])
        nc.gpsimd.dma_start(sin_a[:, ds(3, 2), :], sinT[:, ds(3, 2), :])
        nc.gpsimd.dma_start(wk_a[:, ds(3, 2), :], wkT[:, ds(3, 2), :])
        nc.sync.dma_start(wk_a[:, ds(5, 1), :], wkT[:, ds(5, 1), :])
        nc.scalar.dma_start(sin_a[:, ds(5, 1), :], sinT[:, ds(5, 1), :])
        nc.scalar.dma_start(wv_a[:], wvT[:])
        nc.scalar.dma_start(vbb_t[:], vbb[:])
        nc.scalar.dma_start(wp_a[:], wpT[:])
        nc.gpsimd.dma_start(wq_a[:], wqT[:])
        for qt in range(NQT):
            nc.gpsimd.dma_start(qin_a[:, ds(qt, 1), :, :],
                                qinT[:, ds(qt, 1), :, :])
        nc.gpsimd.dma_start(pbb_t[:], pbb[:])

        for f in range(DC):
            for kt in range(2):
                pp = proj_ps()
                for c in range(DC):
                    nc.tensor.matmul(pp[:, 0:QT],
                                     wk_a[:, ds(c, 1), ts(f, 128)],
                                     sin_a[:, ds(c, 1), ts(kt, QT)],
                                     start=(c == 0), stop=(c == DC - 1))
                nc.vector.tensor_scalar_add(kT[f][:, ts(kt, QT)], pp[:, 0:QT],
                                            kb_t[:, ds(f, 1)])
        for j in range(NKB):
            nc.vector.memset(v_t[j][:], 1.0)
            for w in range(NVW):
                pv = ps.tile([KB, 6, HD], F32, name="pv", tag="proj", bufs=2)
                for c in range(DC):
                    nc.tensor.matmul(pv[:, :, :],
                                     sin_a[:, ds(c, 1), ts(j, KB)],
                                     wv_a[:, ds(c, 1), ts(w, VW)],
                                     start=(c == 0), stop=(c == DC - 1))
                nc.vector.tensor_add(v_t[j][:, ds(w * 6, 6), ds(0, HD)],
                                     pv[:, :, :],
                                     vbb_t[0:KB, ds(w * 6, 6), ds(0, HD)])

    def b_block(qt, f):
        pp = proj_ps()
        for c in range(DC):
            nc.tensor.matmul(pp[:, 0:QT], wq_a[:, ds(c, 1), ts(f, 128)],
                             qin_a[:, ds(qt, 1), ds(c, 1), :],
                             start=(c == 0), stop=(c == DC - 1))
        nc.vector.tensor_scalar_add(qT[f][:, ts(qt, QT)], pp[:, 0:QT],
                                    qb_t[:, ds(f, 1)])

    def d_block(qb):
        qw = min(128, NQ - qb * 128)
        osb = pers.tile([128, DIM], BF16, name="osb", tag="osb", bufs=3)
        for f2 in range(2):
            pp = proj_ps()
            for c in range(DC):
                nc.tensor.matmul(pp[0:qw, 0:VW],
                                 oT[c][:, ds(qb * 128, qw)],
                                 wp_a[:, ds(c, 1), ts(f2, VW)],
                                 start=(c == 0), stop=(c == DC - 1))
            nc.vector.tensor_add(osb[0:qw, ts(f2, VW)], pp[0:qw, 0:VW],
                                 pbb_t[0:qw, ts(f2, VW)])
        nc.gpsimd.dma_start(out_d[ds(qb * 128, qw), :], osb[0:qw, :])

    # ---- phase B(0): Q projection for first q-tile ----
    for f in range(DC):
        b_block(0, f)

    # ---- attention, pipelined over q-tiles ----
    with tc.tile_pool(name="phC", bufs=1) as phC:

        def emit_scores(qt, p):
            """Both heads of the pair in one 2-bank psum tile; the two QK
            matmuls occupy disjoint PE row strips and run concurrently."""
            probs = []
            for j in range(NKB):
                sc = ps.tile([KB, 2, 512], F32, name="sc", tag="sc", bufs=2)
                for e in range(2):
                    off = e * HD
                    nc.tensor.matmul(
                        sc[:, ds(e, 1), ds(0, QT)],
                        kT[p][ds(off, HD), ts(j, KB)],
                        qT[p][ds(off, HD), ts(qt, QT)],
                        start=True, stop=True)
                pr = phC.tile([KB, 2, QT], BF16, name="pr", tag="pr",
                              bufs=14)
                nc.scalar.activation(pr[:, :, :], sc[:, :, ds(0, QT)],
                                     AF.Exp)
                probs.append(pr)
            return probs

        d_done = 0
        blocks = [(qt, p) for qt in range(NQT) for p in range(NP)]
        probs_cur = emit_scores(*blocks[0])
        oU = {}
        for i, (qt, p) in enumerate(blocks):
            # one-block lookahead keeps PE fed while EXP drains this block
            probs_next = emit_scores(*blocks[i + 1]) \
                if i + 1 < len(blocks) else None
            for e in range(2):
                h = 2 * p + e
                po = ps.tile([128, QT], F32, name="po", tag="o", bufs=2)
                for j in range(NKB):
                    nc.tensor.matmul(po[:, :], v_t[j][:, ds(h, 1), :],
                                     probs_cur[j][:, ds(e, 1), :],
                                     start=(j == 0), stop=(j == NKB - 1))
                oU[h] = phC.tile([HD + 1, QT], BF16, name=f"oU{h}",
                                 tag=f"oU{h}", bufs=2)
                nc.vector.tensor_copy(oU[h][:], po[0:HD + 1, :])
                # start the denominator gather as soon as the row exists
                # (parity-major rows: evens 0..5, odds 6..11)
                nc.sync.dma_start(den_b[ds((h % 2) * NP + h // 2, 1), :],
                                  oU[h][ds(HD, 1), :])
            probs_cur = probs_next
            if qt < NQT - 1:
                b_block(qt + 1, p)
            if p < NP - 1:
                continue
            # end of q-tile: one batched reciprocal, then a K=2 block-mask
            # matmul broadcasts both heads of a pair at once
            nc.vector.tensor_copy(den_f[:], den_b[:])
            nc.vector.reciprocal_approx_accurate(rcp_f[:], den_f[:], rcp_s[:])
            nc.vector.tensor_copy(rcp_b[:], rcp_f[:])
            # rows are parity-major, so one DMA per parity row of rbi_t
            for e in range(2):
                nc.gpsimd.dma_start(rbi_t[ds(e, 1), :],
                                    rcp_b[ds(e * NP, NP), :])
            for p2 in range(NP):
                rb = ps.tile([128, 512], F32, name="rb", tag="proj", bufs=2)
                nc.tensor.matmul(rb[:, 0:QT], mask2_t[:],
                                 rbi_t[:, ts(p2, QT)],
                                 start=True, stop=True)
                for e in range(2):
                    h = 2 * p2 + e
                    nc.vector.tensor_mul(
                        oT[p2][ds(e * HD, HD), ts(qt, QT)],
                        oU[h][0:HD, :], rb[ds(e * HD, HD), 0:QT])
            # output projection rows fully covered by finished q-tiles
            d_avail = ((qt + 1) * QT) // 128 if qt < NQT - 1 \
                else math.ceil(NQ / 128)
            while d_done < d_avail:
                d_block(d_done)
                d_done += 1


def build_program():
    from concourse import bacc
    from concourse.compiler_utils import get_compiler_flags, set_compiler_flags
    flags = [f.replace("--enable-ldw-opt=false", "--enable-ldw-opt=true")
             for f in get_compiler_flags()]
    set_compiler_flags(flags)
    nc = bacc.Bacc("TRN2", target_bir_lowering=False, debug=False,
                   num_devices=NCORES)

    def mk(name, shape, dtype=BF16, out=False):
        return nc.dram_tensor(
            name, shape, dtype,
            kind="ExternalOutput" if out else "ExternalInput").ap()

    ins = [
        mk("qinT", [128, NQT, DC, QT]), mk("sinT", [128, DC, NK]),
        mk("wqT", [128, DC, DIM]), mk("wkT", [128, DC, DIM]),
        mk("wvT", [128, DC, DIM]), mk("wpT", [128, DC, DIM]),
        mk("qb2", [128, DC], F32), mk("kb2", [128, DC], F32),
        mk("vbb", [128, NH, HD + 1]), mk("pbb", [128, DIM]),
        mk("mask2", [2, 128]),
    ]
    outs = [mk("out", [NQ, DIM], out=True)]
    with tile.TileContext(nc) as tc:
        with ExitStack() as ctx:
            _emit(ctx, tc, outs, ins)
    nc.compile()
    return nc


def host_prep(inputs):
    """Host-side marshalling: per-core slices, transposed bf16 layouts,
    positional sums folded into the activations, attention scale folded into
    Wq (exact: 0.125 = 2^-3)."""
    import ml_dtypes
    bf = ml_dtypes.bfloat16
    f32 = np.float32
    g = {k: np.asarray(v, dtype=f32) for k, v in inputs.items()}
    t_pat = g["t_x"][1:]                      # (VP, B*T, D)
    s_x = g["s_x"]                            # (AP, B*SPEC, D)

    posq = (g["vmae_space_pos"][:, None, :]
            + g["vmae_temporal_pos"][None, :, :]).reshape(NQ, DIM)
    poss = (g["clip_space_pos"][:, None, :]
            + g["clip_temporal_pos"][None, :, :]).reshape(NK, DIM)

    def pack(a_t):
        # [768, X] -> [128, DC, X]: partition-major so one DMA with big
        # per-partition descriptors loads the whole tensor
        x = a_t.shape[1]
        return np.ascontiguousarray(
            a_t.reshape(DC, 128, x).transpose(1, 0, 2)).astype(bf)

    wqT = pack((SCALE * g["Wq"]).T)
    wkT = pack(g["Wkv"][:DIM].T)
    wvT = pack(g["Wkv"][DIM:].T)
    wpT = pack(g["Wproj"].T)
    qb2 = np.ascontiguousarray((SCALE * g["q_bias"]).reshape(DC, 128).T)
    kb2 = np.ascontiguousarray(g["kv_bias"][:DIM].reshape(DC, 128).T)
    vbb = np.ones((128, NH, HD + 1), dtype=bf)
    vbb[:, :, :HD] = np.tile(
        g["kv_bias"][DIM:].reshape(NH, HD)[None], (128, 1, 1)).astype(bf)
    pbb = np.ascontiguousarray(
        np.tile(g["proj_bias"], (128, 1)).astype(bf))

    mask2 = np.zeros((2, 128), dtype=bf)
    mask2[0, :HD] = 1
    mask2[1, HD:] = 1
    shared = dict(wqT=wqT, wkT=wkT, wvT=wvT, wpT=wpT, qb2=qb2, kb2=kb2,
                  vbb=vbb, pbb=pbb, mask2=mask2)
    in_maps = []
    for b in range(B):
        qin = t_pat[:, b * T:(b + 1) * T, :].reshape(NQ, DIM) + posq
        sin = s_x[:, b * SPEC:(b + 1) * SPEC, :].reshape(NK, DIM) + poss
        qp = pack(qin.T)  # [128, DC, NQ]
        qp = np.ascontiguousarray(
            qp.reshape(128, DC, NQT, QT).transpose(0, 2, 1, 3))
        in_maps.append(dict(
            qinT=qp,
            sinT=pack(sin.T),
            **shared))
    return in_maps


def host_finish(results, t_x):
    o = np.stack([np.asarray(results[b]["out"], dtype=np.float32)
                  for b in range(B)])                  # (B, NQ, D)
    o = o.reshape(B, VP, T, DIM).transpose(1, 0, 2, 3).reshape(VP, B * T, DIM)
    return np.concatenate([np.asarray(t_x, dtype=np.float32)[0:1], o], axis=0)


_NC = None


def kernel(**inputs):
    global _NC
    from concourse.bass_utils import run_bass_kernel_spmd
    if _NC is None:
        _NC = build_program()
    in_maps = host_prep(inputs)
    res = run_bass_kernel_spmd(_NC, in_maps, list(range(NCORES)))
    return host_finish(res.results, inputs["t_x"])
